# revision 1
# baseline (speedup 1.0000x reference)
"""nn_CoMet Trainium2 kernel.

Math (per batch element s in R^16):
  MLP: h1 = logsig(s@W1.T); h2 = h1 + logsig(h1@W2.T); h3 = h2 + logsig(h2@W3.T)
       nnout = h3@W4.T ; d = nnout[:16]; com-rows C = W4[16:20]
  J = d com/d s  (reverse mode through the MLP, 4 rows)
  out = d - J^T (J J^T)^{-1} J d      (== the QR-based projection in the reference)

Kernel design (per core, batch 32768, hidden-major layout [hidden on partitions,
elements on free axis], tiles of 512 elements):
  - forward matmuls in float32r (full-rate fp32-ish PE mode), activations via the
    natural_log_exp table set only:  e=Exp(-a), Lg=Ln(e+1) (= -logsig(a)),
    r=Exp(-Lg), s=e*r (= sig(-a)).  hm_k = -h_k accumulates Lg terms; signs are
    folded into the (host-negated) weights.
  - backward: V accumulates IN PSUM: t3 = s3 @ diag(C_i)W3, then U = (V2*s2)@W2
    accumulated into the same bank; the "+C_i" term is applied on the fly by
    scalar_tensor_tensor (per-partition scalar) when reading PSUM.
  - J and d are produced already transposed (element-major) by using the batch
    tile as the *stationary* matmul operand, so the 4x4 normal-equation solve
    runs on [128 elems, groups, comps] tiles with full lane utilization.
"""

import numpy as np
import ml_dtypes

import concourse.bass as bass
import concourse.mybir as mybir
import concourse.tile as tile

NCORES = 8
B = 262144
BP = B // NCORES          # 32768 per core
NS = 16
NH = 256
NCOM = 4
TN = 512                  # elements per pipeline tile
NTILES = BP // TN         # 64
SUP = 2                   # tiles per supertile (transpose/postproc unit) -> 1024 elems
GRPS = SUP * TN // 128    # 8 groups of 128 elems per supertile
SOLVE_SUPS = 4            # supertiles per batched solve -> 4096 elems
SROWS = SOLVE_SUPS * GRPS # 32

F32 = mybir.dt.float32
F32R = mybir.dt.float32r
BF16 = mybir.dt.bfloat16
AF = mybir.ActivationFunctionType
OP = mybir.AluOpType
AX = mybir.AxisListType

_cache = {}


# ---------------------------------------------------------------- drain patch
def _patch_tile_drain():
    """walrus in this container rejects >1 sem wait on the TileContext final
    drain ("Too many sync wait commands"); split the waits across several
    drain instructions (1 wait each)."""
    if getattr(tile.TileContext, "_comet_patched", False):
        return
    from concourse.vector_clock import ScopedClock

    def _drain_and_barrier(self, tick_clock, wait_clock):
        nc = self.nc
        drain_inst = nc.sync.drain()
        wait_clock.add_sem_waits(
            drain_inst.ins, ScopedClock({None: tick_clock.global_clock})
        )
        si = drain_inst.ins.sync_info
        waits = list(si.on_wait) if si is not None and si.on_wait else []
        if len(waits) > 1:
            si.on_wait = waits[:1]
            for w in waits[1:]:
                extra = nc.sync.drain()
                esi = extra.ins.sync_info
                if esi is None:
                    import bass_rust
                    extra.ins.sync_info = bass_rust.SyncInfo(
                        on_wait=[w], on_update=[]
                    )
                else:
                    esi.on_wait = list(esi.on_wait or []) + [w]
        nc.all_engine_barrier()
        assert self.sems is not None
        popped = nc._tile_sem_poison_stack.pop()
        assert popped is self._sem_poison
        nc.clear_and_free_semaphores(list(self.sems.allocated().values()))
        nc.all_engine_barrier()

    tile.TileContext._drain_and_barrier = _drain_and_barrier
    tile.TileContext._comet_patched = True


def _split_multi_waits(nc):
    """This container's walrus rejects instructions with more than one sync
    wait command.  Hoist extra waits onto injected same-engine NOPs placed
    immediately before the offending instruction."""
    import bass_rust

    for f in nc.m.functions:
        for b in f.blocks:
            insts = list(b.instructions)
            out, dirty = [], False
            for inst in insts:
                si = inst.sync_info
                waits = list(si.on_wait) if si is not None and si.on_wait else []
                if len(waits) > 1:
                    dirty = True
                    for k, wx in enumerate(waits[:-1]):
                        nop = mybir.InstNoOp(name=f"{inst.name}-ws{k}")
                        nop.engine = inst.engine
                        nop.sync_info = bass_rust.SyncInfo(
                            on_wait=[wx], on_update=[])
                        out.append(nop)
                    si.on_wait = waits[-1:]
                out.append(inst)
            if dirty:
                b.instructions = out


# ---------------------------------------------------------------- host prep
def _prep_weights(W1, W2, W3, W4):
    W1 = np.asarray(W1, np.float32)
    W2 = np.asarray(W2, np.float32)
    W3 = np.asarray(W3, np.float32)
    W4 = np.asarray(W4, np.float32)
    W4d, C = W4[:NS], W4[NS:]
    bf = ml_dtypes.bfloat16

    def khalf(a, m):   # [256, m] -> [128, 2, m]
        return np.ascontiguousarray(a.reshape(2, 128, m).transpose(1, 0, 2))

    L1 = np.ascontiguousarray(W1.T)                       # [16,256] lhsT for a1
    L2 = khalf(-W2.T, NH)                                 # [128,2,256]
    L3 = khalf(-W3.T, NH)
    L4 = khalf(-W4d.T, NS)                                # [128,2,16]
    M3 = np.stack([C[i][:, None] * W3 for i in range(4)]) # [4,256,256]
    M3 = np.ascontiguousarray(
        M3.reshape(4, 2, 128, NH).transpose(2, 1, 0, 3)   # [128,2,4,256]
    ).astype(bf)
    W2b = khalf(W2, NH).astype(bf)                        # [128,2,256] lhsT for U
    W1b = khalf(W1, NS).astype(bf)                        # [128,2,16]  rhs for Jt
    Ch = np.ascontiguousarray(C.T.reshape(2, 128, 4).transpose(1, 0, 2)
                              ).reshape(128, 8)           # [128, k*4+i]
    # C as a K=1 lhsT for the ones-matmul: [1, 4i, 256] bf16
    Cb = np.ascontiguousarray(C[:, None, :]).transpose(1, 0, 2).astype(bf)  # [1,4,256]
    return {
        "L1": L1, "L2": L2, "L3": L3, "L4": L4,
        "M3": M3, "W2b": W2b, "W1b": W1b, "Ch": Ch, "Cb": Cb,
    }


# ---------------------------------------------------------------- build
def _build(ntiles=NTILES, a_bufs=1, v_bufs=2):
    _patch_tile_drain()
    nc = bass.Bass()

    zT = nc.dram_tensor("zT", [NS, BP], F32R, kind="ExternalInput")
    dL1 = nc.dram_tensor("L1", [NS, NH], F32R, kind="ExternalInput")
    dL2 = nc.dram_tensor("L2", [128, 2, NH], F32R, kind="ExternalInput")
    dL3 = nc.dram_tensor("L3", [128, 2, NH], F32R, kind="ExternalInput")
    dL4 = nc.dram_tensor("L4", [128, 2, NS], F32R, kind="ExternalInput")
    dM3 = nc.dram_tensor("M3", [128, 2, 4, NH], BF16, kind="ExternalInput")
    dW2b = nc.dram_tensor("W2b", [128, 2, NH], BF16, kind="ExternalInput")
    dW1b = nc.dram_tensor("W1b", [128, 2, NS], BF16, kind="ExternalInput")
    dCh = nc.dram_tensor("Ch", [128, 8], F32, kind="ExternalInput")
    dCb = nc.dram_tensor("Cb", [1, 4, NH], BF16, kind="ExternalInput")
    out_d = nc.dram_tensor("out", [BP, NS], F32, kind="ExternalOutput")

    r32 = lambda ap: ap.bitcast(F32R)

    from contextlib import ExitStack
    with tile.TileContext(nc) as tc, ExitStack() as ctx:
        wpool = ctx.enter_context(tc.tile_pool(name="w", bufs=1))
        sb = ctx.enter_context(tc.tile_pool(name="sb", bufs=3))
        jpool = ctx.enter_context(tc.tile_pool(name="jp", bufs=SOLVE_SUPS + 1))
        mpool = ctx.enter_context(tc.tile_pool(name="mp", bufs=2))
        aps_pool = ctx.enter_context(tc.tile_pool(name="aps", bufs=a_bufs, space="PSUM"))
        vps_pool = ctx.enter_context(tc.tile_pool(name="vps", bufs=v_bufs, space="PSUM"))
        tps_pool = ctx.enter_context(tc.tile_pool(name="tps", bufs=1, space="PSUM"))

        # ---- load constants
        L1s = wpool.tile([NS, NH], F32R)
        nc.sync.dma_start(L1s[:], dL1[:])
        L2s = wpool.tile([128, 2, NH], F32R)
        nc.sync.dma_start(L2s[:], dL2[:])
        L3s = wpool.tile([128, 2, NH], F32R)
        nc.sync.dma_start(L3s[:], dL3[:])
        L4s = wpool.tile([128, 2, NS], F32R)
        nc.sync.dma_start(L4s[:], dL4[:])
        M3s = wpool.tile([128, 2, 4, NH], BF16)
        nc.sync.dma_start(M3s[:], dM3[:])
        W2s = wpool.tile([128, 2, NH], BF16)
        nc.sync.dma_start(W2s[:], dW2b[:])
        W1s = wpool.tile([128, 2, NS], BF16)
        nc.sync.dma_start(W1s[:], dW1b[:])
        Chs = wpool.tile([128, 8], F32)
        nc.sync.dma_start(Chs[:], dCh[:])
        Cbs = wpool.tile([1, 4, NH], BF16)
        nc.sync.dma_start(Cbs[:], dCb[:])
        ones = wpool.tile([1, TN], BF16)
        nc.vector.memset(ones[:], 1.0)

        # persistent per-solve-group state
        jts_list, dts_list = [], []
        msb = vsb = None

        for t in range(ntiles):
            e0 = t * TN
            w = t % SUP          # position within supertile
            s = t // SUP         # supertile index
            sq = s % SOLVE_SUPS  # position within solve group

            if w == 0:
                jt_ps = tps_pool.tile([128, GRPS, 4 * NS], F32, tag="jtps")
                dt_ps = tps_pool.tile([128, GRPS, NS], F32, tag="dtps")
            if sq == 0 and w == 0:
                msb = mpool.tile([128, SROWS, 10], F32, tag="msb")
                vsb = mpool.tile([128, SROWS, 4], F32, tag="vsb")
                csb = mpool.tile([128, SROWS, 4], F32, tag="csb")
                jtg = jpool.tile([128, SROWS, 4 * NS], BF16, tag="jtg")
                dtg = jpool.tile([128, SROWS, NS], F32, tag="dtg")

            # ---------------- forward
            zt = sb.tile([NS, TN], F32R, tag="zt")
            nc.sync.dma_start(zt[:], zT[:, e0:e0 + TN])

            a_ps = aps_pool.tile([128, 2, TN], F32, tag="aps")
            for j in range(2):
                nc.tensor.matmul(a_ps[:, j], L1s[:, j * 128:(j + 1) * 128],
                                 zt[:], start=True, stop=True)

            def act_block(a_psum, layer):
                e = sb.tile([128, 2, TN], F32, tag="e")
                nc.scalar.activation(e[:], a_psum[:], AF.Exp, scale=-1.0)
                lg = sb.tile([128, 2, TN], F32R, tag=f"lg{layer}")
                nc.scalar.activation(lg[:], e[:], AF.Ln, bias=1.0)
                r = sb.tile([128, 2, TN], BF16, tag="r")
                nc.scalar.activation(r[:], lg[:], AF.Exp, scale=-1.0)
                sg = sb.tile([128, 2, TN], BF16, tag=f"s{layer}")
                nc.vector.tensor_scalar(sg[:], r[:], -1.0, 1.0, OP.mult, OP.add)
                return lg, sg

            hm1, s1b = act_block(a_ps, 1)   # hm1 == lg1

            a_ps = aps_pool.tile([128, 2, TN], F32, tag="aps")
            for j in range(2):
                for k in range(2):
                    nc.tensor.matmul(a_ps[:, j],
                                     L2s[:, k, j * 128:(j + 1) * 128],
                                     hm1[:, k], start=(k == 0), stop=(k == 1))
            lg2, s2b = act_block(a_ps, 2)
            hm2 = sb.tile([128, 2, TN], F32R, tag="hm2")
            nc.gpsimd.tensor_tensor(hm2[:], hm1[:], lg2[:], OP.add)

            a_ps = aps_pool.tile([128, 2, TN], F32, tag="aps")
            for j in range(2):
                for k in range(2):
                    nc.tensor.matmul(a_ps[:, j],
                                     L3s[:, k, j * 128:(j + 1) * 128],
                                     hm2[:, k], start=(k == 0), stop=(k == 1))
            lg3, s3b = act_block(a_ps, 3)
            hm3 = sb.tile([128, 2, TN], F32R, tag="hm3")
            nc.gpsimd.tensor_tensor(hm3[:], hm2[:], lg3[:], OP.add)

            # d, transposed: stationary = hm3 chunk, moving = L4 -> [128e, 16]
            for g in range(4):
                grp = w * 4 + g
                for k in range(2):
                    nc.tensor.matmul(dt_ps[:, grp], hm3[:, k, g * 128:(g + 1) * 128],
                                     L4s[:, k], start=(k == 0), stop=(k == 1))

            # ---------------- backward (per com row i), software-pipelined
            # stage A(i): t3+C -> v_ps[i]     (PE)
            # stage B(i): v2s2 = v_ps[i]*s2   (DVE)
            # stage C(i): U += -> v_ps[i]     (PE)
            # stage D(i): v1s1 = v_ps[i]*s1   (DVE)
            # stage E(i): Jt matmuls          (PE)
            v_tiles = [None] * 4
            v2s2_t = [None] * 4
            v1s1_t = [None] * 4

            def stage_a(i):
                v_ps = vps_pool.tile([128, 2, TN], F32, tag="vps", name=f"vps{i}")
                v_tiles[i] = v_ps
                for j in range(2):
                    for k in range(2):
                        nc.tensor.matmul(v_ps[:, j],
                                         M3s[:, k, i, j * 128:(j + 1) * 128],
                                         s3b[:, k], start=(k == 0), stop=False)
                    nc.tensor.matmul(v_ps[:, j], Cbs[:, i, j * 128:(j + 1) * 128],
                                     ones[:], start=False, stop=False)

            def stage_b(i):
                v2s2 = sb.tile([128, 2, TN], BF16, tag="v2s2", name=f"v2s2_{i}")
                v2s2_t[i] = v2s2
                nc.vector.tensor_tensor(v2s2[:], v_tiles[i][:], s2b[:], OP.mult)

            def stage_c(i):
                v_ps = v_tiles[i]
                for j in range(2):
                    for k in range(2):
                        nc.tensor.matmul(v_ps[:, j],
                                         W2s[:, k, j * 128:(j + 1) * 128],
                                         v2s2_t[i][:, k], start=False, stop=(k == 1))

            def stage_d(i):
                v1s1 = sb.tile([128, 2, TN], BF16, tag="v1s1", name=f"v1s1_{i}")
                v1s1_t[i] = v1s1
                nc.vector.tensor_tensor(v1s1[:], v_tiles[i][:], s1b[:], OP.mult)

            def stage_e(i):
                for g in range(4):
                    grp = w * 4 + g
                    for k in range(2):
                        nc.tensor.matmul(jt_ps[:, grp, i * NS:(i + 1) * NS],
                                         v1s1_t[i][:, k, g * 128:(g + 1) * 128],
                                         W1s[:, k], start=(k == 0), stop=(k == 1))

            stages = [stage_a, stage_b, stage_c, stage_d, stage_e]
            for step in range(4 + len(stages) - 1):
                for si in range(len(stages)):
                    i = step - si
                    if 0 <= i < 4:
                        stages[si](i)

            # ---------------- postproc per supertile
            if w == SUP - 1:
                rows = slice(sq * GRPS, (sq + 1) * GRPS)
                jts = jtg[:, rows]
                nc.scalar.copy(jts[:], jt_ps[:])
                dts = dtg[:, rows]
                nc.scalar.copy(dts[:], dt_ps[:])
                jts_list.append(jts)
                dts_list.append(dts)
                pairs = [(0, 0), (0, 1), (0, 2), (0, 3), (1, 1),
                         (1, 2), (1, 3), (2, 2), (2, 3), (3, 3)]
                for idx, (a, b) in enumerate(pairs):
                    prod = sb.tile([128, GRPS, NS], BF16, tag="prod")
                    nc.gpsimd.tensor_tensor(prod[:], jts[:, :, a * NS:(a + 1) * NS],
                                            jts[:, :, b * NS:(b + 1) * NS], OP.mult)
                    nc.vector.tensor_reduce(msb[:, rows, idx:idx + 1], prod[:],
                                            AX.X, OP.add)
                for a in range(4):
                    prod = sb.tile([128, GRPS, NS], F32, tag="prodv")
                    nc.vector.tensor_tensor(prod[:], jts[:, :, a * NS:(a + 1) * NS],
                                            dts[:], OP.mult)
                    nc.vector.tensor_reduce(vsb[:, rows, a:a + 1], prod[:],
                                            AX.X, OP.add)

            # ---------------- batched 4x4 solve + combine, end of solve group
            last = (t == ntiles - 1)
            if w == SUP - 1 and (sq == SOLVE_SUPS - 1 or last):
                nsup = len(jts_list)
                R = nsup * GRPS

                def m(i_):
                    return msb[:, :R, i_:i_ + 1]

                def vv(i_):
                    return vsb[:, :R, i_:i_ + 1]

                tt = {}

                def tmp(name):
                    if name not in tt:
                        tt[name] = mpool.tile([128, SROWS, 1], F32, tag=f"t_{name}", name=f"t_{name}")
                    return tt[name][:, :R]

                V = nc.vector

                def mul(o, x, y):
                    V.tensor_tensor(o, x, y, OP.mult)

                def sub(o, x, y):
                    V.tensor_tensor(o, x, y, OP.subtract)

                def add(o, x, y):
                    V.tensor_tensor(o, x, y, OP.add)

                # index map: 0:00 1:01 2:02 3:03 4:11 5:12 6:13 7:22 8:23 9:33
                m00, m01, m02, m03, m11, m12, m13, m22, m23, m33 = (m(i_) for i_ in range(10))
                x1, x2 = tmp("x1"), tmp("x2")
                detA = tmp("detA")
                mul(x1, m00, m11); mul(x2, m01, m01); sub(detA, x1, x2)
                u10, u11 = tmp("u10"), tmp("u11")
                mul(x1, m11, vv(0)); mul(x2, m01, vv(1)); sub(u10, x1, x2)
                mul(x1, m00, vv(1)); mul(x2, m01, vv(0)); sub(u11, x1, x2)
                P00, P01, P10, P11 = tmp("P00"), tmp("P01"), tmp("P10"), tmp("P11")
                mul(x1, m11, m02); mul(x2, m01, m12); sub(P00, x1, x2)
                mul(x1, m11, m03); mul(x2, m01, m13); sub(P01, x1, x2)
                mul(x1, m00, m12); mul(x2, m01, m02); sub(P10, x1, x2)
                mul(x1, m00, m13); mul(x2, m01, m03); sub(P11, x1, x2)
                S00, S01, S11 = tmp("S00"), tmp("S01"), tmp("S11")
                mul(x1, m02, P00); mul(x2, m12, P10); add(x1, x1, x2)
                mul(S00, detA, m22); sub(S00, S00, x1)
                mul(x1, m02, P01); mul(x2, m12, P11); add(x1, x1, x2)
                mul(S01, detA, m23); sub(S01, S01, x1)
                mul(x1, m03, P01); mul(x2, m13, P11); add(x1, x1, x2)
                mul(S11, detA, m33); sub(S11, S11, x1)
                w0, w1 = tmp("w0"), tmp("w1")
                mul(x1, m02, u10); mul(x2, m12, u11); add(x1, x1, x2)
                mul(w0, detA, vv(2)); sub(w0, w0, x1)
                mul(x1, m03, u10); mul(x2, m13, u11); add(x1, x1, x2)
                mul(w1, detA, vv(3)); sub(w1, w1, x1)
                detS = tmp("detS")
                mul(x1, S00, S11); mul(x2, S01, S01); sub(detS, x1, x2)
                cw2, cw3 = tmp("cw2"), tmp("cw3")
                mul(x1, S11, w0); mul(x2, S01, w1); sub(cw2, x1, x2)
                mul(x1, S00, w1); mul(x2, S01, w0); sub(cw3, x1, x2)
                q0, q1 = tmp("q0"), tmp("q1")
                mul(x1, P00, cw2); mul(x2, P01, cw3); add(x1, x1, x2)
                mul(q0, u10, detS); sub(q0, q0, x1)
                mul(x1, P10, cw2); mul(x2, P11, cw3); add(x1, x1, x2)
                mul(q1, u11, detS); sub(q1, q1, x1)
                dAS, rAS, rS = tmp("dAS"), tmp("rAS"), tmp("rS")
                mul(dAS, detA, detS)
                V.reciprocal(rAS, dAS)
                V.reciprocal(rS, detS)
                mul(csb[:, :R, 0:1], q0, rAS)
                mul(csb[:, :R, 1:2], q1, rAS)
                mul(csb[:, :R, 2:3], cw2, rS)
                mul(csb[:, :R, 3:4], cw3, rS)

                # combine + write out, whole solve group at once
                s_base = s - (nsup - 1)
                R2 = nsup * GRPS
                acc = sb.tile([128, SROWS, NS], F32, tag="acc")
                ctmp = sb.tile([128, SROWS, NS], F32, tag="ctmp")
                G = nc.gpsimd
                for a in range(4):
                    cb = csb[:, :R2, a:a + 1].to_broadcast((128, R2, NS))
                    G.tensor_tensor(ctmp[:, :R2], cb,
                                    jtg[:, :R2, a * NS:(a + 1) * NS], OP.mult)
                    if a == 0:
                        G.tensor_tensor(acc[:, :R2], dtg[:, :R2], ctmp[:, :R2],
                                        OP.subtract)
                    else:
                        G.tensor_tensor(acc[:, :R2], acc[:, :R2], ctmp[:, :R2],
                                        OP.subtract)
                eb = s_base * SUP * TN
                nc.sync.dma_start(
                    out_d[eb:eb + R2 * 128, :].rearrange(
                        "(g p) m -> p g m", p=128),
                    acc[:, :R2])
                jts_list, dts_list = [], []

    _split_multi_waits(nc)
    return nc


# ---------------------------------------------------------------- entry point
def kernel(zstates, W1, W2, W3, W4):
    from concourse.bass_utils import run_bass_kernel_spmd

    key = "full"
    if key not in _cache:
        _cache[key] = _build()
    nc = _cache[key]

    wm = _prep_weights(W1, W2, W3, W4)
    z = np.asarray(zstates, np.float32).reshape(NCORES, BP, NS)
    in_maps = [
        {**wm, "zT": np.ascontiguousarray(z[c].T)} for c in range(NCORES)
    ]
    res = run_bass_kernel_spmd(nc, in_maps, core_ids=list(range(NCORES)))
    return np.concatenate([res.results[c]["out"] for c in range(NCORES)], axis=0)

